# revision 47
# baseline (speedup 1.0000x reference)
"""Multi-head self-attention Trainium2 kernel (8-core SPMD).

Problem: B=4, S=2048, E=1024, 16 heads x 64 dim, int mask, softmax attention.

Sharding: core c handles batch b=c//2 and head-half hh=c%2 (8 heads).
Each core computes Yp = Attn(X[b])[:, heads(hh)] @ wO[rows(hh)]  -> [S, E]
partial product; host sums the two partials per batch and adds bO.

v4 schedule: the softmax exp stream on the ACT engine (256 x [128,1024],
~267us) is the target to saturate; every other engine hides under it.
  - Engines execute in program order, so projections are emitted
    JUST-IN-TIME inside the attention pipeline: pair p's K^T/Q^T chunks at
    the head of pair p's k-loop (kt chunk g right before the k-tiles that
    read it), V chunks inside pair 0's k-loop, phase-3 Y for q-chunk qc
    inside chunk qc+1's first pair.
  - The k-loop is software-pipelined: PV(k) is emitted after scores(k+1),
    and each pair's PV(15) + normalization are deferred into the NEXT
    pair's k=1 slot (finish-carry), so the next pair's first scores/exp
    never wait behind the previous pair's tail.
  - ACT runs ONLY exp; all PSUM evacuations (QK/V/Y, PV staging) run on
    DVE with QK bias fused in; ~half the mask multiplies run on GPSIMD.
  - PSUM (8 banks): ps_sc holds score tiles [128,1024] AND the [128,512]
    projection/V/Y groups in a 3-slot ring (6 banks); ps_pv holds the two
    PV accumulators [65,512] (ones column = rowsum, 2 banks).
  - The mask streams per q-chunk (16 KB/part, 2 bufs); input DMAs split
    across the SP and ACT HWDGE queues; reciprocal rowsums broadcast
    across partitions via a DRAM round-trip DMA. The last q-chunk
    processes pairs in reverse so the final Y only waits on pair 0.
"""

import sys

if "/opt/trn_rl_repo" not in sys.path:
    sys.path.insert(0, "/opt/trn_rl_repo")

import numpy as np
import ml_dtypes

import concourse.bass as bass
import concourse.tile as tile
from concourse import bacc, mybir
from concourse.bass_utils import run_bass_kernel_spmd

F32 = mybir.dt.float32
BF16 = mybir.dt.bfloat16
AF = mybir.ActivationFunctionType

S = 2048      # sequence length
E = 1024      # embed dim
DH = 512      # d_all per core (8 heads x 64)
D = 64        # head dim
H = 8         # heads per core
NE = 8        # embed 128-tiles
ND = 4        # d_all 128-tiles (= head pairs)
NS = 16       # seq 128-tiles
NK = 16       # k 128-tiles
NQC = 4       # q chunks of 512
QC = 512      # q chunk width
V1W = D + 1   # V columns per head incl. ones column


def _emit(nc, tc, ctx, d):
    P = 128
    glob = ctx.enter_context(tc.tile_pool(name="glob", bufs=1))

    qt = glob.tile([P, ND * S], BF16)    # QT: [r, p*2048+q], d_all = 128p+r
    kt = glob.tile([P, ND * S], BF16)
    v1 = glob.tile([P, NS * H * V1W], BF16)  # V1: [s%128, st*520 + h*65 + j]
    bq = glob.tile([P, ND], F32)
    bk = glob.tile([P, ND], F32)
    bvb = glob.tile([P, DH], F32)        # bV broadcast across partitions
    otn = glob.tile([P, ND * S], BF16)   # normalized out^T
    wo = glob.tile([P, ND * E], BF16)    # wO: [r, p*1024+c], d_all = 128p+r

    xt = glob.tile([P, NE * S], BF16)    # X^T: [r, e*2048+s], embed = 128e+r
    wq = glob.tile([P, NE * DH], BF16)   # wQ: [r, e*512+c]
    wk = glob.tile([P, NE * DH], BF16)
    wv = glob.tile([P, NE * DH], BF16)

    # PSUM (8 banks): ps_sc = score tiles [128,1024] + [128,512] proj/V/Y
    # groups, 3 slots x 2 banks; ps_pv = PV accumulators, 2 x 1 bank.
    ps_sc = ctx.enter_context(tc.tile_pool(name="ps_sc", bufs=3, space="PSUM"))
    ps_pv = ctx.enter_context(tc.tile_pool(name="ps_pv", bufs=2, space="PSUM"))

    # DMA priority order, split across two HWDGE queues (SP + ACT).
    # First matmuls need wK/wQ + XT; biases are only needed by the first
    # PSUM evacuations, so they come after.
    # only the dd0 column slices of wK/wQ gate the first projections;
    # the remaining columns follow after the XT chunks
    nc.sync.dma_start(
        wk[:].rearrange("p (e c) -> p e c", c=DH)[:, :, 0:P],
        d["wK"].ap().rearrange("(e p) c -> p e c", p=P)[:, :, 0:P],
    )
    nc.scalar.dma_start(
        wq[:].rearrange("p (e c) -> p e c", c=DH)[:, :, 0:P],
        d["wQ"].ap().rearrange("(e p) c -> p e c", p=P)[:, :, 0:P],
    )
    for e in range(NE):
        eng = nc.sync if e % 2 == 0 else nc.scalar
        eng.dma_start(
            xt[:, e * S:(e + 1) * S],
            d["XT"].ap().rearrange("(e p) s -> e p s", p=P)[e],
        )
    nc.scalar.dma_start(
        wv[:].rearrange("p (e c) -> p e c", c=DH),
        d["wV"].ap().rearrange("(e p) c -> p e c", p=P),
    )
    nc.sync.dma_start(bq[:], d["bQ"].ap().rearrange("(n p) -> p n", p=P))
    nc.sync.dma_start(bk[:], d["bK"].ap().rearrange("(n p) -> p n", p=P))
    nc.sync.dma_start(
        bvb[:], d["bV"].ap().rearrange("(a s) -> a s", a=1).partition_broadcast(P)
    )
    nc.sync.dma_start(
        wk[:].rearrange("p (e c) -> p e c", c=DH)[:, :, P:DH],
        d["wK"].ap().rearrange("(e p) c -> p e c", p=P)[:, :, P:DH],
    )
    nc.scalar.dma_start(
        wq[:].rearrange("p (e c) -> p e c", c=DH)[:, :, P:DH],
        d["wQ"].ap().rearrange("(e p) c -> p e c", p=P)[:, :, P:DH],
    )

    # mask streamed per q-chunk: [128, 16*512] bf16 (16 KB/part), 2 bufs.
    mtpool = ctx.enter_context(tc.tile_pool(name="mtq", bufs=2))
    mtq = {}

    def prefetch_mask(qc):
        if qc >= NQC:
            return
        # one strided-AP DMA for all 16 k-slices: keeps the SP DMA queue
        # short so the per-pair normalization round-trips aren't delayed
        t = mtpool.tile([P, NK * QC], BF16, tag="m")
        nc.sync.dma_start(
            t[:].rearrange("p (k q) -> p k q", k=NK),
            d["maskT"].ap().rearrange("(k p) q -> p k q", p=P)[
                :, :, qc * QC:(qc + 1) * QC],
        )
        mtq[qc] = t

    prefetch_mask(0)
    nc.scalar.dma_start(
        wo[:].rearrange("p (n c) -> p n c", c=E),
        d["wO"].ap().rearrange("(n p) c -> p n c", p=P),
    )
    prefetch_mask(1)

    # ones columns of V1 (before V writes; disjoint columns)
    nc.vector.memset(
        v1[:].rearrange("p (t h j) -> p t h j", t=NS, j=V1W)[:, :, :, D:D + 1],
        1.0,
    )
    onesb = glob.tile([1, D], F32)   # ones row for the tail PE broadcast
    nc.vector.memset(onesb[:], 1.0)

    def proj_qk(w_sb, out_t, b_t, dd, sc):
        # one [128,512] chunk of QT/KT = sum_e w[e, dd]^T XT[e, sc]; bias
        # fused into the DVE PSUM evacuation.
        ps = ps_sc.tile([P, 512], F32, tag="sc")
        for e in range(NE):
            nc.tensor.matmul(
                ps[:],
                w_sb[:, e * DH + dd * P: e * DH + (dd + 1) * P],
                xt[:, e * S + sc * 512: e * S + sc * 512 + 512],
                start=(e == 0), stop=(e == NE - 1),
            )
        nc.vector.tensor_scalar_add(
            out_t[:, dd * S + sc * 512: dd * S + sc * 512 + 512],
            ps[:], b_t[:, dd:dd + 1],
        )

    def v_chunk(st):
        # V[s, c] = sum_e XT[e, s] * wV[e, c] for one seq 128-tile
        ps = ps_sc.tile([P, 512], F32, tag="sc")
        for e in range(NE):
            nc.tensor.matmul(
                ps[:],
                xt[:, e * S + st * P: e * S + (st + 1) * P],
                wv[:, e * DH:(e + 1) * DH],
                start=(e == 0), stop=(e == NE - 1),
            )
        dst = v1[:, st * H * V1W:(st + 1) * H * V1W].rearrange(
            "p (h j) -> p h j", j=V1W
        )[:, :, 0:D]
        nc.vector.tensor_add(
            dst,
            ps[:].rearrange("p (h j) -> p h j", j=D),
            bvb[:].rearrange("p (h j) -> p h j", j=D),
        )

    with (
        tc.tile_pool(name="p2str", bufs=5) as p2str,
        tc.tile_pool(name="p2nrm", bufs=3) as p2nrm,
        tc.tile_pool(name="p2dram", bufs=4, space="DRAM") as p2dram,
        tc.tile_pool(name="p3sb", bufs=4) as p3sb,
    ):
        def attn(qc, p, pre_k=None, carry=None, last=False):
            """Attention for (q-chunk qc, head pair p).

            pre_k(k): emits just-in-time projection work before k-tile k.
            carry: finish() of the previous pair, invoked at k==1 so the
            previous pair's PV(15) + normalization hide under this pair's
            exp stream. Returns this pair's finish(). last=True swaps the
            DRAM-round-trip reciprocal broadcast for a rank-1 PE matmul
            broadcast (low latency; the final Y emission waits on it)."""
            pv1 = ps_pv.tile([V1W, QC], F32, tag="pv")
            pv2 = ps_pv.tile([V1W, QC], F32, tag="pv")
            qoff = p * S + qc * QC
            prs = {}

            def pv_step(k):
                pr = prs.pop(k)
                for h, pv in ((0, pv1), (1, pv2)):
                    head = 2 * p + h
                    nc.tensor.matmul(
                        pv[:],
                        v1[:, (k * H + head) * V1W: (k * H + head) * V1W + V1W],
                        pr[:, h * QC:(h + 1) * QC],
                        start=(k == 0), stop=(k == NK - 1),
                    )

            for k in range(NK):
                if pre_k is not None:
                    pre_k(k)
                s = ps_sc.tile([P, 1024], F32, tag="sc")
                for h in (0, 1):
                    # row-tiled pair: head h on PE rows h*64..h*64+63
                    nc.tensor.matmul(
                        s[:, h * QC:(h + 1) * QC],
                        kt[h * D:(h + 1) * D, p * S + k * P: p * S + (k + 1) * P],
                        qt[h * D:(h + 1) * D, qoff:qoff + QC],
                        start=True, stop=True,
                    )
                e = p2str.tile([P, 1024], BF16, tag="es")
                nc.scalar.activation(e[:], s[:], AF.Exp)
                pr = p2str.tile([P, 1024], BF16, tag="pr")
                mv = mtq[qc][:, k * QC:(k + 1) * QC]
                nc.vector.tensor_mul(pr[:, 0:QC], e[:, 0:QC], mv)
                eng = nc.gpsimd if (k % 2 == 1 and k < 14) else nc.vector
                eng.tensor_mul(pr[:, QC:2 * QC], e[:, QC:2 * QC], mv)
                prs[k] = pr
                if k == 1 and carry is not None:
                    carry()
                if k > 0:
                    pv_step(k - 1)   # software pipeline: PV one k behind

            def finish():
                pv_step(NK - 1)
                # stage PV out of PSUM, then normalize via a DRAM
                # round-trip broadcast of the reciprocal rowsums (row D).
                st = p2nrm.tile([P, QC], BF16, tag="st")
                nc.vector.tensor_copy(st[0:D, :], pv1[0:D, :])
                nc.vector.tensor_copy(st[D:P, :], pv2[0:D, :])
                rs = p2nrm.tile([P, 2 * QC], F32, tag="rs")
                if last:
                    # reciprocal to partition 0, then broadcast down 64
                    # partitions via ones[1,64]^T (x) recip on the PE
                    nc.vector.reciprocal(rs[0:1, 0:QC], pv1[D:D + 1, :])
                    nc.vector.reciprocal(rs[0:1, QC:2 * QC], pv2[D:D + 1, :])
                    rbp = ps_sc.tile([P, 1024], F32, tag="sc")
                    nc.tensor.matmul(rbp[0:D, 0:QC], onesb[:], rs[0:1, 0:QC],
                                     start=True, stop=True)
                    nc.tensor.matmul(rbp[0:D, QC:2 * QC], onesb[:],
                                     rs[0:1, QC:2 * QC], start=True, stop=True)
                    nc.vector.tensor_mul(
                        otn[0:D, qoff:qoff + QC], st[0:D, :], rbp[0:D, 0:QC]
                    )
                    nc.vector.tensor_mul(
                        otn[D:P, qoff:qoff + QC], st[D:P, :],
                        rbp[0:D, QC:2 * QC]
                    )
                    return
                nc.vector.reciprocal(rs[D:D + 1, 0:QC], pv1[D:D + 1, :])
                nc.vector.reciprocal(rs[D:D + 1, QC:2 * QC], pv2[D:D + 1, :])
                dsc1 = p2dram.tile([1, QC], F32, tag="d1")
                dsc2 = p2dram.tile([1, QC], F32, tag="d2")
                nc.sync.dma_start(dsc1[:], rs[D:D + 1, 0:QC])
                nc.sync.dma_start(dsc2[:], rs[D:D + 1, QC:2 * QC])
                rb = p2nrm.tile([P, QC], F32, tag="rb")
                nc.sync.dma_start(rb[0:D, :], dsc1[:].partition_broadcast(D))
                nc.sync.dma_start(rb[D:P, :], dsc2[:].partition_broadcast(D))
                nc.vector.tensor_mul(
                    otn[0:D, qoff:qoff + QC], st[0:D, :], rb[0:D, :]
                )
                nc.vector.tensor_mul(
                    otn[D:P, qoff:qoff + QC], st[D:P, :], rb[D:P, :]
                )

            return finish

        def emit_y_group(qc, g):
            # one (q-tile, E-half) group of Y = out.T @ wO for chunk qc
            qi, ec = qc * 4 + g // 2, g % 2
            yps = ps_sc.tile([P, 512], F32, tag="sc")
            for p in range(ND):
                nc.tensor.matmul(
                    yps[:],
                    otn[:, p * S + qi * P: p * S + (qi + 1) * P],
                    wo[:, p * E + ec * 512: p * E + ec * 512 + 512],
                    start=(p == 0), stop=(p == ND - 1),
                )
            ysb = p3sb.tile([P, 512], BF16, tag="ys")
            nc.vector.tensor_copy(ysb[:], yps[:])
            nc.sync.dma_start(
                d["Yp"].ap()[qi * P:(qi + 1) * P, ec * 512:(ec + 1) * 512],
                ysb[:],
            )

        def emit_y(qc):
            for g in range(8):
                emit_y_group(qc, g)

        # ---- q-chunk 0: projections woven into the attention pipeline ----
        proj_qk(wk, kt, bk, 0, 0)
        proj_qk(wq, qt, bq, 0, 0)

        def pre_k_pair0(k):
            # V chunk st must exist before PV(st), which runs at k=st+1;
            # front-load slightly so V stays ahead of the PV pipeline.
            # The NEXT pair's chunk-0 projections emit at k=13/14 so its
            # first exp never waits on projection matmuls.
            if k > 0 and k % 4 == 0:
                proj_qk(wk, kt, bk, 0, k // 4)   # kt dd0 chunk k//4
            if k == 1:
                v_chunk(0)          # PV(st) runs at k=st+1; chunk st must
            elif k == 2:
                v_chunk(1)          # exist by then -- spread the emission
                v_chunk(2)          # so no single slot delays the exps
            elif 3 <= k <= NS - 1:
                v_chunk(k)
            if k == 9:
                proj_qk(wk, kt, bk, 1, 0)
            elif k == 11:
                proj_qk(wq, qt, bq, 1, 0)

        def mk_pre_k(dd, extra_qt):
            # all look-ahead work sits in mid-pair slots (never k>=13) so
            # the PE queue is clear of projections at the pair boundary
            def pre_k(k):
                if k > 0 and k % 4 == 0:
                    proj_qk(wk, kt, bk, dd, k // 4)
                elif k == 9 and dd + 1 < ND:
                    proj_qk(wk, kt, bk, dd + 1, 0)
                elif k == 11 and dd + 1 < ND:
                    proj_qk(wq, qt, bq, dd + 1, 0)
                elif k in (3, 6, 10):
                    # trailing qt chunks for later q-chunks (dd_prev, sc 1-3)
                    proj_qk(wq, qt, bq, extra_qt, 1 + (3, 6, 10).index(k))
            return pre_k

        fin = attn(0, 0, pre_k=pre_k_pair0)
        for p in range(1, ND):
            fin = attn(0, p, pre_k=mk_pre_k(p, p - 1), carry=fin)

        def mk_pre_k_y(qc_prev):
            # interleave the previous chunk's 8 Y groups into this pair's
            # k-loop so they never stall the exp stream
            def pre_k(k):
                if k < 8:
                    emit_y_group(qc_prev, k)
            return pre_k

        def pre_k_qt3(k):
            if k in (3, 6, 10):
                proj_qk(wq, qt, bq, 3, 1 + (3, 6, 10).index(k))

        for qc in range(1, NQC):
            prefetch_mask(qc + 1)
            pairs = list(range(ND)) if qc < NQC - 1 else list(range(ND - 1, -1, -1))
            for i, p in enumerate(pairs):
                if i == 1:
                    pk = mk_pre_k_y(qc - 1)
                elif qc == 1 and i == 0:
                    pk = pre_k_qt3
                else:
                    pk = None
                fin = attn(qc, p, pre_k=pk, carry=fin,
                           last=(qc == NQC - 1 and i == ND - 1))
        fin()
        emit_y(NQC - 1)


def build_module(reps=1):
    from contextlib import ExitStack

    nc = bacc.Bacc("TRN2", target_bir_lowering=False, debug=False)
    d = {
        "XT": nc.dram_tensor("XT", [E, S], BF16, kind="ExternalInput"),
        "maskT": nc.dram_tensor("maskT", [S, S], BF16, kind="ExternalInput"),
        "wQ": nc.dram_tensor("wQ", [E, DH], BF16, kind="ExternalInput"),
        "wK": nc.dram_tensor("wK", [E, DH], BF16, kind="ExternalInput"),
        "wV": nc.dram_tensor("wV", [E, DH], BF16, kind="ExternalInput"),
        "wO": nc.dram_tensor("wO", [DH, E], BF16, kind="ExternalInput"),
        "bQ": nc.dram_tensor("bQ", [DH], F32, kind="ExternalInput"),
        "bK": nc.dram_tensor("bK", [DH], F32, kind="ExternalInput"),
        "bV": nc.dram_tensor("bV", [DH], F32, kind="ExternalInput"),
        "Yp": nc.dram_tensor("Yp", [S, E], BF16, kind="ExternalOutput"),
    }
    with tile.TileContext(nc) as tc:
        for _ in range(reps):
            with ExitStack() as ctx:
                _emit(nc, tc, ctx, d)
    nc.compile()
    return nc


def make_in_maps(X, mask, wQ, bQ, wK, bK, wV, bV, wO, bO):
    """Per-core input dicts. Core c: batch c//2, head-half c%2."""
    in_maps = []
    for c in range(8):
        b, hh = c // 2, c % 2
        cols = slice(hh * DH, (hh + 1) * DH)
        in_maps.append({
            "XT": np.ascontiguousarray(np.asarray(X[b]).T).astype(ml_dtypes.bfloat16),
            "maskT": np.ascontiguousarray(
                np.asarray(mask[b, 0]).T
            ).astype(ml_dtypes.bfloat16),
            "wQ": (np.asarray(wQ[:, cols]) * np.float32(0.125)).astype(ml_dtypes.bfloat16),
            "wK": np.asarray(wK[:, cols]).astype(ml_dtypes.bfloat16),
            "wV": np.asarray(wV[:, cols]).astype(ml_dtypes.bfloat16),
            "wO": np.asarray(wO[cols, :]).astype(ml_dtypes.bfloat16),
            "bQ": np.ascontiguousarray(np.asarray(bQ[cols])) * np.float32(0.125),
            "bK": np.ascontiguousarray(np.asarray(bK[cols])),
            "bV": np.ascontiguousarray(np.asarray(bV[cols])),
        })
    return in_maps


_NC = None


def kernel(X, mask, wQ, bQ, wK, bK, wV, bV, wO, bO):
    global _NC
    if _NC is None:
        _NC = build_module()
    in_maps = make_in_maps(X, mask, wQ, bQ, wK, bK, wV, bV, wO, bO)
    res = run_bass_kernel_spmd(_NC, in_maps, list(range(8)))
    B = 4
    Y = np.empty((B, S, E), dtype=np.float32)
    bO = np.asarray(bO, dtype=np.float32)
    for b in range(B):
        Y[b] = (res.results[2 * b]["Yp"].astype(np.float32)
                + res.results[2 * b + 1]["Yp"].astype(np.float32) + bO)
    return Y
